# revision 12
# baseline (speedup 1.0000x reference)
"""Cox partial-likelihood loss on 8 Trainium2 NeuronCores.

reference:
    theta = hazard_pred.reshape(-1)                 # [n]
    R[i, j] = survtime[j] >= survtime[i]            # risk-set mask
    risk_sum[i] = sum_j exp(theta[j]) * R[i, j]
    loss = -mean((theta - log(risk_sum)) * censor)

Sharding: rows i are split across 8 cores (1024 rows each). Each core
computes its [8192 x 1024] slice of e_j-weighted risk mask in 64 chunks
of 128 j's and contracts each chunk on the TensorEngine into PSUM.

v2 design (vs v1's DVE/ACT Sign-correction scheme):
  - survtime is cast to fp16 on the host; the DVE tensor_scalar
    (s_i <= s_j) * e_j runs with all-2-byte tensor operands, which
    engages the DVE 4x perf mode (~330ns/chunk vs 684 at 2x).
    Comparing fp16(s_i) <= fp16(s_j) keeps the diagonal exact, so no
    sign-correction machinery is needed at all; fp16 ties/rounding
    perturb the loss by ~1e-3 relative, well inside the 2e-2 gate.
  - The mask tile already carries e_j (scalar2 = per-partition fp32
    e column, exempt from the 2-byte rule), so every matmul uses the
    same constant ones[128,1] stationary vector: risk_sum[i] = P[i].
  - With mask production at ~330ns/chunk and PE consumption at
    ~432ns/chunk, the PE never starves, stays in continuous
    execution, and ramps to its 2.4 GHz p-state (the v1 kernel sat at
    the ~1.2 GHz mid p-state boundary, which is why it measured
    ~52-59us instead of its ~28us engine-busy floor).

j-index mapping: j = p*64 + c (p = SBUF partition, c = chunk column),
so survtime/theta load as contiguous [128, 64] tiles and chunk c uses
column c for the per-partition compare/weight scalars.

Host sums the 8 partial row-sums and applies -1/n.
"""

import sys
from contextlib import ExitStack, nullcontext

import numpy as np

try:  # concourse ships with the container toolchain, not on sys.path by default
    import concourse  # noqa: F401
except ImportError:
    sys.path.insert(0, "/opt/trn_rl_repo")

import concourse.bacc as bacc
import concourse.bass as bass
import concourse.tile as tile
from concourse import mybir
from concourse.bass_utils import run_bass_kernel_spmd

DT = mybir.dt
AF = mybir.ActivationFunctionType
N = 8192
CORES = 8
NL = N // CORES       # 1024 local rows per core
NCHUNK = 64           # j-chunks of 128
NHALF = NL // 2       # matmul free-dim limit is 512 (PSUM bank)

MASK_BUFS = 7
SKEW = 1              # p0 trails p1 by SKEW chunks: consecutive PE matmuls
                      # then read different mask tiles (removes a same-tile
                      # back-to-back stall) and p1 closes early so its
                      # Ln/reduce overlap the last p0 matmuls. Swept on HW:
                      # SKEW=1 30.0us, 2 48.9, 4 34.0, 8 38.5 (1-core).
SIB_MODE = "hw4"  # 4-way HWDGE split broadcast

_CACHE: dict = {}


def _emit_body(nc, const, masks, psums, tailp, st32r_all, th_all, st16_loc,
               th_loc, cen_loc, partial):
    # j-major tiles: [p, c] holds index j = p*64 + c
    # st32r holds fp16-rounded survtime in fp32 (compare scalars must be
    # fp32); values match si16's fp16 rounding exactly, so the diagonal
    # i==j compare is a true tie and every row keeps its self-term.
    st32r = const.tile([128, NCHUNK], DT.float32)
    nc.sync.dma_start(out=st32r, in_=st32r_all[:].rearrange("(p c) -> p c", c=NCHUNK))
    th_sb = const.tile([128, NCHUNK], DT.float32)
    nc.sync.dma_start(out=th_sb, in_=th_all[:].rearrange("(p c) -> p c", c=NCHUNK))

    e32 = const.tile([128, NCHUNK], DT.float32)
    nc.scalar.activation(out=e32, in_=th_sb, func=AF.Exp)

    # tail inputs (DMAs early; dependent compute emitted after the loop)
    thl = tailp.tile([1, NL], DT.float32)
    nc.sync.dma_start(out=thl, in_=th_loc[:].rearrange("(o n) -> o n", o=1))
    cenl = tailp.tile([1, NL], DT.float32)
    nc.sync.dma_start(out=cenl, in_=cen_loc[:].rearrange("(o n) -> o n", o=1))

    # local survtime (fp16) broadcast to all partitions (free dim = row i)
    si16 = const.tile([128, NL], DT.float16)
    st_loc_row = st16_loc[:].rearrange("(o n) -> o n", o=1)
    for q in range(4):
        nc.sync.dma_start(
            out=si16[q * 32 : (q + 1) * 32, :],
            in_=st_loc_row.partition_broadcast(32),
        )

    ones16 = const.tile([128, 1], DT.float16)
    nc.vector.memset(ones16, 1.0)

    # dummy Ln pre-loads the Ln activation table early so the tail Ln
    # isn't stalled on a ~1.3us table load
    onesf = tailp.tile([1, 1], DT.float32)
    nc.vector.memset(onesf, 1.0)
    ln_warm = tailp.tile([1, 1], DT.float32)
    nc.scalar.activation(out=ln_warm, in_=onesf, func=AF.Ln)
    # theta*censor product off the critical path on gpsimd
    thc = tailp.tile([1, NL], DT.float32)
    nc.gpsimd.tensor_mul(thc, thl, cenl)

    # main loop: P[i] accumulates sum_j e_j * (s_i <= s_j) via PE.
    # p1 consumes chunk c at slot c; p0 trails by SKEW so p1 closes
    # early and its Ln+reduce overlap the final p0 matmuls.
    p0 = psums.tile([1, NHALF], DT.float32, tag="p0")
    p1 = psums.tile([1, NHALF], DT.float32, tag="p1")
    lnt = tailp.tile([1, NL], DT.float32)
    lnc = tailp.tile([1, NL], DT.float32)
    sum1 = tailp.tile([1, 1], DT.float32)
    tiles = {}
    for c in range(NCHUNK + SKEW):
        if c < NCHUNK:
            m = masks.tile([128, NL], DT.float16, tag="m")
            tiles[c] = m
            nc.vector.tensor_scalar(
                out=m,
                in0=si16,
                scalar1=st32r[:, c : c + 1],
                scalar2=e32[:, c : c + 1],
                op0=mybir.AluOpType.is_le,
                op1=mybir.AluOpType.mult,
            )
            nc.tensor.matmul(
                p1, ones16, m[:, NHALF:NL], start=(c == 0),
                stop=(c == NCHUNK - 1),
            )
        if c == NCHUNK:
            # p1 closed: start its tail while p0 finishes the last chunks
            nc.scalar.activation(out=lnt[:, NHALF:NL], in_=p1, func=AF.Ln)
            nc.vector.tensor_mul(lnc[:, NHALF:NL], lnt[:, NHALF:NL],
                                 cenl[:, NHALF:NL])
            nc.vector.tensor_reduce(
                out=sum1, in_=lnc[:, NHALF:NL], axis=mybir.AxisListType.X,
                op=mybir.AluOpType.add,
            )
        if c >= SKEW:
            cc = c - SKEW
            nc.tensor.matmul(
                p0, ones16, tiles.pop(cc)[:, 0:NHALF], start=(cc == 0),
                stop=(cc == NCHUNK - 1),
            )

    # tail: risk = P ; partial = sum(theta*cen) - sum(ln(risk)*cen)
    nc.scalar.activation(out=lnt[:, 0:NHALF], in_=p0, func=AF.Ln)
    lc_sum = tailp.tile([1, 1], DT.float32)
    nc.vector.tensor_mul(lnc[:, 0:NHALF], lnt[:, 0:NHALF], cenl[:, 0:NHALF])
    nc.vector.tensor_reduce(
        out=lc_sum, in_=lnc[:, 0:NHALF], axis=mybir.AxisListType.X,
        op=mybir.AluOpType.add,
    )
    thc_sum = tailp.tile([1, 1], DT.float32)
    nc.vector.tensor_reduce(
        out=thc_sum, in_=thc, axis=mybir.AxisListType.X, op=mybir.AluOpType.add
    )
    res = tailp.tile([1, 1], DT.float32)
    nc.vector.tensor_sub(res, thc_sum, lc_sum)
    nc.vector.tensor_sub(res, res, sum1)
    nc.sync.dma_start(out=partial[:].rearrange("(o n) -> o n", o=1), in_=res)


def _build_nc(reps: int | None = None) -> bass.Bass:
    nc = bacc.Bacc()
    st32r_all = nc.declare_dram_parameter("st32r_all", [N], DT.float32, isOutput=False)
    th_all = nc.declare_dram_parameter("th_all", [N], DT.float32, isOutput=False)
    st16_loc = nc.declare_dram_parameter("st16_loc", [NL], DT.float16, isOutput=False)
    th_loc = nc.declare_dram_parameter("th_loc", [NL], DT.float32, isOutput=False)
    cen_loc = nc.declare_dram_parameter("cen_loc", [NL], DT.float32, isOutput=False)
    partial = nc.declare_dram_parameter("partial", [1], DT.float32, isOutput=True)

    with tile.TileContext(nc) as tc, ExitStack() as ctx:
        const = ctx.enter_context(tc.tile_pool(name="const", bufs=1))
        masks = ctx.enter_context(tc.tile_pool(name="masks", bufs=MASK_BUFS))
        psums = ctx.enter_context(tc.tile_pool(name="psums", bufs=1, space="PSUM"))
        tailp = ctx.enter_context(tc.tile_pool(name="tailp", bufs=1))

        loop = (
            tc.For_i(0, reps, 1,
                     hint_engines=(mybir.EngineType.PE, mybir.EngineType.DVE))
            if reps is not None
            else nullcontext()
        )
        with loop:
            _emit_body(nc, const, masks, psums, tailp, st32r_all, th_all,
                       st16_loc, th_loc, cen_loc, partial)

    nc.compile()
    return nc


def _get_nc() -> bass.Bass:
    if "nc" not in _CACHE:
        _CACHE["nc"] = _build_nc()
    return _CACHE["nc"]


def make_in_maps(survtime: np.ndarray, theta: np.ndarray, censor: np.ndarray):
    st = np.ascontiguousarray(survtime, dtype=np.float32)
    st16 = st.astype(np.float16)
    st32r = st16.astype(np.float32)
    th = np.ascontiguousarray(theta, dtype=np.float32).reshape(-1)
    cen = np.ascontiguousarray(censor, dtype=np.float32)
    in_maps = []
    for k in range(CORES):
        lo, hi = k * NL, (k + 1) * NL
        in_maps.append(
            {
                "st32r_all": st32r,
                "th_all": th,
                "st16_loc": st16[lo:hi].copy(),
                "th_loc": th[lo:hi].copy(),
                "cen_loc": cen[lo:hi].copy(),
            }
        )
    return in_maps


def kernel(hazard_pred: np.ndarray, survtime: np.ndarray, censor: np.ndarray):
    nc = _get_nc()
    in_maps = make_in_maps(survtime, hazard_pred, censor)
    out = run_bass_kernel_spmd(nc, in_maps, list(range(CORES)))
    partials = np.array(
        [np.asarray(out.results[k]["partial"]).reshape(-1)[0] for k in range(CORES)],
        dtype=np.float64,
    )
    return np.float32(-partials.sum() / N)
